# revision 8
# baseline (speedup 1.0000x reference)
"""DCMC contrastive-loss kernel for 8 Trainium2 NeuronCores (SPMD, Bass/Tile).

Row-sharded strategy (512 rows/core):
  Phase 1 — six FCNs (3 target + 3 momentum-blended online) level-major in a
            transposed [out_ch(part), row(free)] layout; full-batch BN stats via
            3 batched AllReduces; 3 decoder MLPs; L2-normalized target features
            AllGathered in d-major and row-major layouts.
  Phase 2 — per view: E = exp(20*G - 20) slab in [key(part), query(free)] layout;
            rs1 via PE-ones reduce; H = E @ fbn on PE; each contrastive term's
            exp-logit row-sum (SE) comes free from ACT accum_out during PSUM
            eviction and its DP dot is a tiny DVE op against H.
  Host    — loss_row = log(SE) - 2*dpraw/(|q|*rs1); the similarity-matrix
            "adaptive" correction is exactly zero unless an off-diagonal
            similarity exceeds ~0.3 (detected via rs1/Eii with huge margin);
            then, or when warm_up != 0, a faithful numpy fallback runs.
"""
import os
import sys

sys.path.insert(0, '/opt/trn_rl_repo')

import numpy as np
import ml_dtypes

from concourse import bacc, tile, mybir, bass_utils

BF16 = ml_dtypes.bfloat16
BF = mybir.dt.bfloat16
F32 = mybir.dt.float32
AF = mybir.ActivationFunctionType
ALU = mybir.AluOpType

N = 4096
NCORES = 8
R = N // NCORES            # 512 rows per core
DIMS = [1024, 1024, 512, 256]
NV = 3
TEMP = 0.5
BN_EPS = 1e-5
NTILES = [d // 128 for d in DIMS[1:]]   # [8, 4, 2] out tiles per level
D = DIMS[-1]               # 256 = feature dim
DK = D // 128              # 2 partition chunks of the feature dim
JT = N // 128              # 32 key tiles per view

# terms: (q_matrix_index, key_view); q-matrix 0-2 = fa_v, 3-5 = Q_v
TERMS = [(0, 0), (1, 1), (2, 2),
         (3, 1), (3, 2), (4, 0), (4, 2), (5, 0), (5, 1)]

_PROGRAM = None
LAST_EXEC_NS = None


def _build_program():
    nc = bacc.Bacc("TRN2", target_bir_lowering=False, debug=False,
                   enable_asserts=False, num_devices=NCORES)

    # ---------------- DRAM I/O ----------------
    xT = [nc.dram_tensor(f"xT{v}", [DIMS[0], R], BF, kind="ExternalInput")
          for v in range(NV)]
    W = [[nc.dram_tensor(f"w{f}_{l}", [DIMS[l], DIMS[l + 1]], BF,
                         kind="ExternalInput") for l in range(3)]
         for f in range(6)]
    gam = [nc.dram_tensor(f"gam{l}", [128, 6 * NTILES[l]], F32,
                          kind="ExternalInput") for l in range(3)]
    bet = [nc.dram_tensor(f"bet{l}", [128, 6 * NTILES[l]], F32,
                          kind="ExternalInput") for l in range(3)]
    wd1 = [nc.dram_tensor(f"wd1{v}", [D, 4 * D], BF, kind="ExternalInput")
           for v in range(NV)]
    wd2 = [nc.dram_tensor(f"wd2{v}", [4 * D, D], BF, kind="ExternalInput")
           for v in range(NV)]
    b1 = [nc.dram_tensor(f"b1{v}", [128, 8], F32, kind="ExternalInput")
          for v in range(NV)]
    b2 = [nc.dram_tensor(f"b2{v}", [128, 2], F32, kind="ExternalInput")
          for v in range(NV)]
    eye_in = nc.dram_tensor("eye_in", [128, 128], BF, kind="ExternalInput")

    outA = nc.dram_tensor("outA", [R, 24], F32, kind="ExternalOutput")
    outB = nc.dram_tensor("outB", [8, R], F32, kind="ExternalOutput")
    dbg_fbn = nc.dram_tensor("dbg_fbn", [D, R], F32, kind="ExternalOutput")
    dbg_fa = nc.dram_tensor("dbg_fa", [D, R], F32, kind="ExternalOutput")
    dbg_q = nc.dram_tensor("dbg_q", [D, R], F32, kind="ExternalOutput")

    rg = [list(range(NCORES))]

    with tile.TileContext(nc) as tc:
        with tc.tile_pool(name="px", bufs=1) as px, \
             tc.tile_pool(name="pdram", bufs=1, space="DRAM") as pdram:
            # constants
            ones128 = px.tile([128, 1], BF, tag="ones128", name="ones128")
            nc.gpsimd.memset(ones128[:], 1.0)
            ones1f = px.tile([1, 128], F32, tag="ones1f", name="ones1f")
            nc.gpsimd.memset(ones1f[:], 1.0)
            bias_m20 = px.tile([128, 1], F32, tag="bias_m20", name="bias_m20")
            nc.gpsimd.memset(bias_m20[:], -20.0)
            bias_eps = px.tile([128, 1], F32, tag="bias_eps", name="bias_eps")
            nc.gpsimd.memset(bias_eps[:], BN_EPS)
            bias_ln2 = px.tile([128, 1], F32, tag="bias_ln2", name="bias_ln2")
            nc.gpsimd.memset(bias_ln2[:], float(np.log(2.0)))
            eye = px.tile([128, 128], BF, tag="eye", name="eye")
            nc.sync.dma_start(eye[:], eye_in[:])

            outA_t = [px.tile([128, 24], F32, tag=f"outA{i}", name=f"outA{i}") for i in range(4)]
            for i in range(4):
                nc.gpsimd.memset(outA_t[i][:], 0.0)

            # ------------- Phase 1: FCNs in two groups (target, online) ------
            X = [None] * 6          # per-FCN activations (k-major tiles)

            def fcn_group(grp):
                fcn_ids = [3 * grp, 3 * grp + 1, 3 * grp + 2]
                act_cm = tc.tile_pool(name=f"actg{grp}", bufs=1)
                act_pool = act_cm.__enter__()
                try:
                    for l in range(3):
                        ntl = NTILES[l]
                        nk = DIMS[l] // 128
                        final = (l == 2)
                        next_pool = px if final else act_pool
                        with tc.tile_pool(name=f"lvl{grp}_{l}", bufs=1) as lvl, \
                             tc.tile_pool(name=f"scr{grp}_{l}", bufs=3) as scr, \
                             tc.tile_pool(name=f"pw{grp}_{l}", bufs=DIMS[l] // 128 + 2) as pw, \
                             tc.tile_pool(name=f"pps{grp}_{l}", bufs=3,
                                          space="PSUM") as pps:
                            if l == 0:
                                x0 = {}
                                for v in range(NV):
                                    tiles = []
                                    for k in range(nk):
                                        xt = lvl.tile([128, R], BF,
                                                      tag=f"x0_{v}_{k}",
                                                      name=f"x0_{v}_{k}")
                                        nc.sync.dma_start(
                                            xt[:],
                                            xT[v][128 * k:128 * (k + 1), :])
                                        tiles.append(xt)
                                    x0[v] = tiles
                            WD = 3 * ntl
                            s12 = lvl.tile([128, 2 * WD], F32, tag="s12",
                                           name="s12")
                            yb = {}
                            for fi, f in enumerate(fcn_ids):
                                xin = x0[f % 3] if l == 0 else X[f]
                                wsl = []
                                for k in range(nk):
                                    wt = pw.tile([128, DIMS[l + 1]], BF,
                                                 tag="wsl", name="wsl")
                                    nc.sync.dma_start(
                                        wt[:], W[f][l][128 * k:128 * (k + 1), :])
                                    wsl.append(wt)
                                ybf = []
                                for m in range(ntl):
                                    ps = pps.tile([128, R], F32, tag="mm",
                                                  name="mm")
                                    for k in range(nk):
                                        nc.tensor.matmul(
                                            ps[:],
                                            wsl[k][:, 128 * m:128 * (m + 1)],
                                            xin[k][:],
                                            start=(k == 0), stop=(k == nk - 1))
                                    col = fi * ntl + m
                                    t = lvl.tile([128, R], BF,
                                                 tag=f"yb{fi}_{m}",
                                                 name=f"yb{fi}_{m}")
                                    nc.scalar.activation(
                                        t[:], ps[:], AF.Copy,
                                        accum_out=s12[:, col:col + 1])
                                    sqs = scr.tile([128, R], BF, tag="sq",
                                                   name="sq")
                                    nc.scalar.activation(
                                        sqs[:], ps[:], AF.Square,
                                        accum_out=s12[:, WD + col:WD + col + 1])
                                    ybf.append(t)
                                yb[f] = ybf
                            # batched AllReduce of this group+level's stats
                            ar_in = pdram.tile([128, 2 * WD], F32,
                                               tag=f"arin{grp}_{l}",
                                               name=f"arin{grp}_{l}")
                            ar_out = pdram.tile([128, 2 * WD], F32,
                                                tag=f"arout{grp}_{l}",
                                                name=f"arout{grp}_{l}")
                            nc.sync.dma_start(ar_in[:], s12[:])
                            nc.gpsimd.collective_compute(
                                "AllReduce", ALU.add, replica_groups=rg,
                                ins=[ar_in.opt()], outs=[ar_out.opt()])
                            s12r = lvl.tile([128, 2 * WD], F32, tag="s12r",
                                            name="s12r")
                            nc.sync.dma_start(s12r[:], ar_out[:])
                            mu = lvl.tile([128, WD], F32, tag="mu", name="mu")
                            nc.vector.tensor_scalar_mul(mu[:], s12r[:, 0:WD],
                                                        1.0 / N)
                            ex2 = lvl.tile([128, WD], F32, tag="ex2", name="ex2")
                            nc.vector.tensor_scalar_mul(ex2[:],
                                                        s12r[:, WD:2 * WD],
                                                        1.0 / N)
                            var = lvl.tile([128, WD], F32, tag="var", name="var")
                            nc.vector.tensor_tensor(out=var[:], in0=mu[:],
                                                    in1=mu[:], op=ALU.mult)
                            nc.vector.tensor_tensor(out=var[:], in0=ex2[:],
                                                    in1=var[:], op=ALU.subtract)
                            lnv = lvl.tile([128, WD], F32, tag="lnv", name="lnv")
                            nc.scalar.activation(lnv[:], var[:], AF.Ln,
                                                 bias=bias_eps[:])
                            rsd = lvl.tile([128, WD], F32, tag="rsd", name="rsd")
                            nc.scalar.activation(rsd[:], lnv[:], AF.Exp,
                                                 scale=-0.5)
                            gml = lvl.tile([128, WD], F32, tag="gml", name="gml")
                            nc.sync.dma_start(
                                gml[:], gam[l][:, WD * grp:WD * (grp + 1)])
                            btl = lvl.tile([128, WD], F32, tag="btl", name="btl")
                            nc.sync.dma_start(
                                btl[:], bet[l][:, WD * grp:WD * (grp + 1)])
                            sc = lvl.tile([128, WD], F32, tag="sc", name="sc")
                            nc.vector.tensor_tensor(out=sc[:], in0=gml[:],
                                                    in1=rsd[:], op=ALU.mult)
                            sh = lvl.tile([128, WD], F32, tag="sh", name="sh")
                            nc.vector.tensor_tensor(out=sh[:], in0=mu[:],
                                                    in1=sc[:], op=ALU.mult)
                            nc.vector.tensor_tensor(out=sh[:], in0=btl[:],
                                                    in1=sh[:], op=ALU.subtract)
                            func = AF.Relu if l < 2 else AF.Identity
                            for fi, f in enumerate(fcn_ids):
                                xo = []
                                for m in range(ntl):
                                    col = fi * ntl + m
                                    tag = (f"X{l + 1}_{f}_{m}" if not final
                                           else f"feat_{f}_{m}")
                                    t = next_pool.tile([128, R], BF, tag=tag,
                                                       name=tag)
                                    nc.scalar.activation(
                                        t[:], yb[f][m][:], func,
                                        scale=sc[:, col:col + 1],
                                        bias=sh[:, col:col + 1])
                                    xo.append(t)
                                X[f] = xo
                finally:
                    act_cm.__exit__(None, None, None)

            fcn_group(0)   # target FCNs first: their AllGathers can launch early
            fbT = {v: X[v] for v in range(NV)}     # target features (d-major)

            fbn = {}
            q_im = [[None] * 4 for _ in range(6)]
            sesc = [[None] * 4 for _ in range(6)]
            # ------- l2norm of target features, Eii, AllGathers, transposes ----
            ag1_in = pdram.tile([NV * D, R], BF, tag="ag1in", name="ag1in")
            ag1_out = pdram.tile([NCORES * NV * D, R], BF, tag="ag1out", name="ag1out")
            ag2_in = pdram.tile([R, NV * D], BF, tag="ag2in", name="ag2in")
            ag2_out = pdram.tile([N, NV * D], BF, tag="ag2out", name="ag2out")
            with tc.tile_pool(name="pns", bufs=3) as pns, \
                 tc.tile_pool(name="ppsn", bufs=1, space="PSUM") as ppsn, \
                 tc.tile_pool(name="pptr", bufs=2, space="PSUM") as pptr:
                for v in range(NV):
                    nps = ppsn.tile([1, R], F32, tag="nps", name="nps")
                    for k in range(DK):
                        sq = pns.tile([128, R], BF, tag="sqn", name="sqn")
                        nc.vector.tensor_tensor(out=sq[:], in0=fbT[v][k][:],
                                                in1=fbT[v][k][:], op=ALU.mult)
                        nc.tensor.matmul(nps[:], ones128[:], sq[:],
                                         start=(k == 0), stop=(k == DK - 1))
                    lnn = pns.tile([1, R], F32, tag="lnn", name="lnn")
                    nc.scalar.activation(lnn[:], nps[:], AF.Ln)
                    invn = pns.tile([1, R], F32, tag="invn", name="invn")
                    nc.scalar.activation(invn[:], lnn[:], AF.Exp, scale=-0.5)
                    bcp = ppsn.tile([128, R], F32, tag="bcp", name="bcp")
                    nc.tensor.matmul(bcp[:], ones1f[:], invn[:],
                                     start=True, stop=True)
                    ft = []
                    for k in range(DK):
                        t = px.tile([128, R], BF, tag=f"fbn{v}_{k}", name=f"fbn{v}_{k}")
                        nc.vector.tensor_tensor(out=t[:], in0=fbT[v][k][:],
                                                in1=bcp[:], op=ALU.mult)
                        ft.append(t)
                        nc.sync.dma_start(
                            ag1_in[D * v + 128 * k:D * v + 128 * (k + 1), :],
                            t[:])
                    fbn[v] = ft
                    # Eii = exp(20*||fbn_i||^2 - 20) for the host trigger check
                    gps = ppsn.tile([1, R], F32, tag="gps", name="gps")
                    for k in range(DK):
                        sq = pns.tile([128, R], BF, tag="sqn", name="sqn")
                        nc.vector.tensor_tensor(out=sq[:], in0=ft[k][:],
                                                in1=ft[k][:], op=ALU.mult)
                        nc.tensor.matmul(gps[:], ones128[:], sq[:],
                                         start=(k == 0), stop=(k == DK - 1))
                    eii = pns.tile([1, R], F32, tag="eii", name="eii")
                    nc.scalar.activation(eii[:], gps[:], AF.Exp, scale=20.0,
                                         bias=bias_m20[0:1, :])
                    nc.sync.dma_start(outB[3 + v:4 + v, :], eii[:])

                nc.gpsimd.collective_compute(
                    "AllGather", ALU.bypass, replica_groups=rg,
                    ins=[ag1_in.opt()], outs=[ag1_out.opt()])

                # row-major copies of fbn for the 2nd AllGather
                for v in range(NV):
                    for i in range(4):
                        fim = pns.tile([128, D], BF, tag="fim", name="fim")
                        for k in range(DK):
                            tp = pptr.tile([128, 128], BF, tag="trp", name="trp")
                            nc.tensor.transpose(
                                tp[:], fbn[v][k][:, 128 * i:128 * (i + 1)],
                                eye[:])
                            nc.scalar.activation(
                                fim[:, 128 * k:128 * (k + 1)], tp[:], AF.Copy)
                        nc.sync.dma_start(
                            ag2_in[128 * i:128 * (i + 1), D * v:D * (v + 1)],
                            fim[:])
                nc.gpsimd.collective_compute(
                    "AllGather", ALU.bypass, replica_groups=rg,
                    ins=[ag2_in.opt()], outs=[ag2_out.opt()])


            fcn_group(1)   # online FCNs
            faT = {v: X[3 + v] for v in range(NV)}  # online features (d-major)

            # ------------- MLP decoders -------------
            QT = {}
            with tc.tile_pool(name="pmlp", bufs=1) as pmlp, \
                 tc.tile_pool(name="pwd", bufs=10) as pwd, \
                 tc.tile_pool(name="ppsm", bufs=3, space="PSUM") as ppsm:
                for v in range(NV):
                    wsl = []
                    for k in range(DK):
                        wt = pwd.tile([128, 4 * D], BF, tag="wd1", name="wd1")
                        nc.sync.dma_start(wt[:],
                                          wd1[v][128 * k:128 * (k + 1), :])
                        wsl.append(wt)
                    b1t = pmlp.tile([128, 8], F32, tag=f"b1{v}", name=f"b1{v}")
                    nc.sync.dma_start(b1t[:], b1[v][:])
                    b2t = pmlp.tile([128, 2], F32, tag=f"b2{v}", name=f"b2{v}")
                    nc.sync.dma_start(b2t[:], b2[v][:])
                    h = []
                    for m in range(8):
                        ps = ppsm.tile([128, R], F32, tag="mmh", name="mmh")
                        for k in range(DK):
                            nc.tensor.matmul(
                                ps[:], wsl[k][:, 128 * m:128 * (m + 1)],
                                faT[v][k][:],
                                start=(k == 0), stop=(k == DK - 1))
                        ht = pmlp.tile([128, R], BF, tag=f"h{m}", name=f"h{m}")
                        nc.scalar.activation(ht[:], ps[:], AF.Relu,
                                             bias=b1t[:, m:m + 1])
                        h.append(ht)
                    wsl2 = []
                    for k in range(8):
                        wt = pwd.tile([128, D], BF, tag="wd2", name="wd2")
                        nc.sync.dma_start(wt[:],
                                          wd2[v][128 * k:128 * (k + 1), :])
                        wsl2.append(wt)
                    qt = []
                    for m in range(DK):
                        ps = ppsm.tile([128, R], F32, tag="mmh", name="mmh")
                        for k in range(8):
                            nc.tensor.matmul(
                                ps[:], wsl2[k][:, 128 * m:128 * (m + 1)],
                                h[k][:], start=(k == 0), stop=(k == 7))
                        t = px.tile([128, R], BF, tag=f"QT{v}_{m}", name=f"QT{v}_{m}")
                        nc.scalar.activation(t[:], ps[:], AF.Identity,
                                             bias=b2t[:, m:m + 1])
                        qt.append(t)
                    QT[v] = qt

            qmats = [faT[0], faT[1], faT[2], QT[0], QT[1], QT[2]]
            with tc.tile_pool(name="pns2", bufs=3) as pns, \
                 tc.tile_pool(name="pptr2", bufs=2, space="PSUM") as pptr:
                # i-major query copies + seScale (2/|q|)
                for m in range(6):
                    for i in range(4):
                        qim = px.tile([128, D], BF, tag=f"qim{m}_{i}", name=f"qim{m}_{i}")
                        for k in range(DK):
                            tp = pptr.tile([128, 128], BF, tag="trp", name="trp")
                            nc.tensor.transpose(
                                tp[:], qmats[m][k][:, 128 * i:128 * (i + 1)],
                                eye[:])
                            nc.scalar.activation(
                                qim[:, 128 * k:128 * (k + 1)], tp[:], AF.Copy)
                        q_im[m][i] = qim
                        sqs = pns.tile([128, D], BF, tag="sqq", name="sqq")
                        nc.scalar.activation(
                            sqs[:], qim[:], AF.Square,
                            accum_out=outA_t[i][:, 18 + m:19 + m])
                        lnq = pns.tile([128, 1], F32, tag="lnq", name="lnq")
                        nc.scalar.activation(lnq[:],
                                             outA_t[i][:, 18 + m:19 + m], AF.Ln)
                        ssc = px.tile([128, 1], F32, tag=f"sesc{m}_{i}", name=f"sesc{m}_{i}")
                        nc.scalar.activation(ssc[:], lnq[:], AF.Exp, scale=-0.5,
                                             bias=bias_ln2[:])
                        sesc[m][i] = ssc

                # debug outputs (view 0)
                for k in range(DK):
                    for src, dst in ((fbn[0][k], dbg_fbn), (faT[0][k], dbg_fa),
                                     (QT[0][k], dbg_q)):
                        df = pns.tile([128, R], F32, tag="dbg", name="dbg")
                        nc.scalar.activation(df[:], src[:], AF.Copy)
                        nc.sync.dma_start(dst[128 * k:128 * (k + 1), :], df[:])

            # ------------- Phase 2: per-view N x N work -------------
            with tc.tile_pool(name="pfk", bufs=2) as pfk, \
                 tc.tile_pool(name="pjm", bufs=4) as pjm, \
                 tc.tile_pool(name="pE", bufs=1) as pE, \
                 tc.tile_pool(name="pscr", bufs=3) as pscr, \
                 tc.tile_pool(name="ph", bufs=1) as ph, \
                 tc.tile_pool(name="ppe", bufs=2, space="PSUM") as ppe, \
                 tc.tile_pool(name="pph", bufs=1, space="PSUM") as pph, \
                 tc.tile_pool(name="ppq", bufs=3, space="PSUM") as ppq:
                for v in range(NV):
                    fkT = []
                    for k in range(DK):
                        t = pfk.tile([128, N], BF, tag=f"fkT{k}", name=f"fkT{k}")
                        for c in range(NCORES):
                            base = NV * D * c + D * v + 128 * k
                            nc.sync.dma_start(t[:, R * c:R * (c + 1)],
                                              ag1_out[base:base + 128, :])
                        fkT.append(t)
                    # E slab + rs1 + H (software-pipelined PE emission)
                    rs1_ps = pph.tile([1, R], F32, tag="rs1ps", name="rs1ps")
                    H_ps = pph.tile([128, 4 * D], F32, tag="hps", name="hps")
                    Es, jms = [None] * JT, [None] * JT

                    def red_step(jt):
                        nc.tensor.matmul(rs1_ps[:], ones128[:], Es[jt][:],
                                         start=(jt == 0), stop=(jt == JT - 1),
                                         skip_group_check=True)
                        for i in range(4):
                            nc.tensor.matmul(
                                H_ps[:, D * i:D * (i + 1)],
                                Es[jt][:, 128 * i:128 * (i + 1)], jms[jt][:],
                                start=(jt == 0), stop=(jt == JT - 1),
                                skip_group_check=True)

                    for jt in range(JT):
                        fkjm = pjm.tile([128, D], BF, tag="fkjm", name="fkjm")
                        nc.sync.dma_start(
                            fkjm[:],
                            ag2_out[128 * jt:128 * (jt + 1), D * v:D * (v + 1)])
                        jms[jt] = fkjm
                        eps_t = ppe.tile([128, R], F32, tag="eps", name="eps")
                        for k in range(DK):
                            nc.tensor.matmul(
                                eps_t[:], fkT[k][:, 128 * jt:128 * (jt + 1)],
                                fbn[v][k][:],
                                start=(k == 0), stop=(k == DK - 1))
                        E_t = pE.tile([128, R], BF, tag=f"E{jt}", name=f"E{jt}")
                        nc.scalar.activation(E_t[:], eps_t[:], AF.Exp,
                                             scale=20.0, bias=bias_m20[:])
                        Es[jt] = E_t
                        if jt > 0:
                            red_step(jt - 1)
                    red_step(JT - 1)
                    rs1_sb = pscr.tile([1, R], F32, tag="rs1sb", name="rs1sb")
                    nc.scalar.activation(rs1_sb[:], rs1_ps[:], AF.Copy)
                    nc.sync.dma_start(outB[v:v + 1, :], rs1_sb[:])
                    H_sb = []
                    for i in range(4):
                        t = ph.tile([128, D], F32, tag=f"hsb{i}", name=f"hsb{i}")
                        nc.scalar.activation(t[:], H_ps[:, D * i:D * (i + 1)],
                                             AF.Copy)
                        H_sb.append(t)
                    # q slabs
                    tlist = [(t, m) for t, (m, kv) in enumerate(TERMS)
                             if kv == v]
                    for (t_idx, m) in tlist:
                        qT = qmats[m]
                        for i in range(4):
                            seacc = pscr.tile([128, 8], F32, tag="seacc", name="seacc")
                            for jb in range(8):
                                qps = ppq.tile([128, R], F32, tag="qps", name="qps")
                                for k in range(DK):
                                    nc.tensor.matmul(
                                        qps[:],
                                        qT[k][:, 128 * i:128 * (i + 1)],
                                        fkT[k][:, R * jb:R * (jb + 1)],
                                        start=(k == 0), stop=(k == DK - 1))
                                scr2 = pscr.tile([128, R], BF, tag="scr2", name="scr2")
                                nc.scalar.activation(
                                    scr2[:], qps[:], AF.Exp,
                                    scale=sesc[m][i][:],
                                    accum_out=seacc[:, jb:jb + 1])
                            nc.vector.tensor_reduce(
                                outA_t[i][:, t_idx:t_idx + 1], seacc[:],
                                axis=mybir.AxisListType.X, op=ALU.add)
                            dtt = pscr.tile([128, D], F32, tag="dtt", name="dtt")
                            nc.vector.tensor_tensor(out=dtt[:],
                                                    in0=H_sb[i][:],
                                                    in1=q_im[m][i][:],
                                                    op=ALU.mult)
                            nc.vector.tensor_reduce(
                                outA_t[i][:, 9 + t_idx:10 + t_idx], dtt[:],
                                axis=mybir.AxisListType.X, op=ALU.add)

                zrow = pscr.tile([1, R], F32, tag="zrow", name="zrow")
                nc.gpsimd.memset(zrow[:], 0.0)
                nc.sync.dma_start(outB[6:7, :], zrow[:])
                nc.sync.dma_start(outB[7:8, :], zrow[:])
                for i in range(4):
                    nc.sync.dma_start(outA[128 * i:128 * (i + 1), :],
                                      outA_t[i][:])

    nc.compile()
    return nc


# ======================= host side =======================

def _tmap(fn, *ts):
    t0 = ts[0]
    if isinstance(t0, dict):
        return {k: _tmap(fn, *[t[k] for t in ts]) for k in t0}
    if isinstance(t0, (list, tuple)):
        return [_tmap(fn, *xs) for xs in zip(*ts)]
    return fn(*ts)


def _pack_chan(vec, ntl):
    # [128*ntl] -> [128, ntl] with channel o at [o % 128, o // 128]
    return np.ascontiguousarray(vec.reshape(ntl, 128).T.astype(np.float32))


def _prep_in_maps(data, online_new, target, decoder):
    base = {}
    for f in range(6):
        p = target[f] if f < 3 else online_new[f - 3]
        for l in range(3):
            base[f"w{f}_{l}"] = np.ascontiguousarray(p['Ws'][l].astype(BF16))
    for l in range(3):
        ntl = NTILES[l]
        g = np.empty((128, 6 * ntl), np.float32)
        b = np.empty((128, 6 * ntl), np.float32)
        for f in range(6):
            p = target[f] if f < 3 else online_new[f - 3]
            if l < 2:
                gv, bv = np.asarray(p['gammas'][l]), np.asarray(p['betas'][l])
            else:
                gv = np.ones(DIMS[3], np.float32)
                bv = np.zeros(DIMS[3], np.float32)
            g[:, f * ntl:(f + 1) * ntl] = _pack_chan(gv, ntl)
            b[:, f * ntl:(f + 1) * ntl] = _pack_chan(bv, ntl)
        base[f"gam{l}"] = g
        base[f"bet{l}"] = b
    for v in range(NV):
        dp = decoder[v]
        base[f"wd1{v}"] = np.ascontiguousarray(dp['W1'].astype(BF16))
        base[f"wd2{v}"] = np.ascontiguousarray(dp['W2'].astype(BF16))
        base[f"b1{v}"] = _pack_chan(np.asarray(dp['b1'], np.float32), 8)
        base[f"b2{v}"] = _pack_chan(np.asarray(dp['b2'], np.float32), 2)
    base["eye_in"] = np.eye(128, dtype=BF16)
    in_maps = []
    for c in range(NCORES):
        m = dict(base)
        for v in range(NV):
            sl = data[v][R * c:R * (c + 1), :]
            m[f"xT{v}"] = np.ascontiguousarray(sl.T).astype(BF16)
        in_maps.append(m)
    return in_maps


def _l2n(x):
    return x / np.maximum(np.linalg.norm(x, axis=1, keepdims=True), 1e-12)


def _reference_numpy(data, online_new, target, decoder, warm_up):
    """Faithful (f32-matrix / f64-reduction) replication of the reference."""
    def fcn(x, p):
        for i in range(2):
            y = x @ p['Ws'][i]
            mu = y.mean(0, dtype=np.float64).astype(np.float32)
            var = ((y - mu) ** 2).mean(0, dtype=np.float64).astype(np.float32)
            y = (y - mu) / np.sqrt(var + BN_EPS) * p['gammas'][i] + p['betas'][i]
            x = np.maximum(y, 0)
        y = x @ p['Ws'][2]
        mu = y.mean(0, dtype=np.float64).astype(np.float32)
        var = ((y - mu) ** 2).mean(0, dtype=np.float64).astype(np.float32)
        return (y - mu) / np.sqrt(var + BN_EPS)

    def mlp(x, p):
        return np.maximum(x @ p['W1'] + p['b1'], 0) @ p['W2'] + p['b2']

    def cal_sim(feat, temp=0.1):
        f = _l2n(feat)
        n = f.shape[0]
        euc = np.clip(2.0 - 2.0 * (f @ f.T), 0.0, None).astype(np.float32)
        sim = np.exp(-euc / np.float32(temp))
        sim = sim / sim.sum(1, keepdims=True)
        dg = np.diag(sim).copy()
        diff = np.abs(dg[:, None] - sim)
        thresh = (diff < 0.7).astype(np.float32)
        idx = np.argmin(diff + np.eye(n, dtype=np.float32), axis=1)
        possible = np.eye(n, dtype=np.float32)
        possible[np.arange(n), idx] += 1.0
        selected = thresh * possible
        sim_exp = np.exp(sim) * (1.0 - np.eye(n, dtype=np.float32))
        weight = 1.0 - sim_exp / sim_exp.sum(1, keepdims=True)
        adaptive = selected * weight + (1.0 - selected)
        return sim * adaptive

    def contrast(q, k, mask):
        logits = (_l2n(q) @ _l2n(k).T) / np.float32(TEMP)
        mx = logits.max(1, keepdims=True)
        lse = mx + np.log(np.exp(logits - mx).sum(1, keepdims=True))
        s = lse - logits
        return float((np.sum(s * mask, axis=1, dtype=np.float64)
                      / mask.sum(1, dtype=np.float64)).sum() / (N * N))

    fa = [fcn(data[v], online_new[v]) for v in range(NV)]
    Q = [mlp(fa[v], decoder[v]) for v in range(NV)]
    fb = [fcn(data[v], target[v]) for v in range(NV)]
    if warm_up:
        ne = [np.eye(N, dtype=np.float32)] * NV
    else:
        ne = [cal_sim(fb[v]) for v in range(NV)]
    li = (contrast(fa[0], fb[0], ne[0]) + contrast(fa[1], fb[1], ne[1])
          + contrast(fa[2], fb[2], ne[2])) / 3.0
    le = (contrast(Q[0], fb[1], ne[1]) + contrast(Q[0], fb[2], ne[2])
          + contrast(Q[1], fb[0], ne[0]) + contrast(Q[1], fb[2], ne[2])
          + contrast(Q[2], fb[0], ne[0]) + contrast(Q[2], fb[1], ne[1])) / 6.0
    return np.float32(li + le)


def kernel(data0, data1, data2, online_params, target_params, decoder_params,
           momentum, warm_up):
    global _PROGRAM, LAST_EXEC_NS
    data = [np.asarray(d, np.float32) for d in (data0, data1, data2)]
    f32 = lambda x: np.asarray(x, np.float32)
    online = _tmap(f32, list(online_params))
    target = _tmap(f32, list(target_params))
    decoder = _tmap(f32, list(decoder_params))
    mom = np.float32(momentum)
    online_new = _tmap(lambda a, b: mom * a + (np.float32(1) - mom) * b,
                       online, target)
    wu = int(np.asarray(warm_up))
    if wu:
        return np.asarray(
            _reference_numpy(data, online_new, target, decoder, wu))

    if _PROGRAM is None:
        _PROGRAM = _build_program()
    nc = _PROGRAM
    in_maps = _prep_in_maps(data, online_new, target, decoder)
    trace = os.environ.get("BASS_KERNEL_PROFILE", "0") == "1"
    res = bass_utils.run_bass_kernel_spmd(
        nc, in_maps, core_ids=list(range(NCORES)), trace=trace)
    LAST_EXEC_NS = res.exec_time_ns

    A = np.stack([res.results[c]["outA"] for c in range(NCORES)])  # [8,512,24]
    B = np.stack([res.results[c]["outB"] for c in range(NCORES)])  # [8,8,512]
    rs1 = B[:, 0:3, :].astype(np.float64)          # [core, view, row]
    eii = B[:, 3:6, :].astype(np.float64)
    if (not np.all(np.isfinite(A)) or not np.all(np.isfinite(B))
            or np.any(rs1 <= 0)
            or np.any((rs1 - eii) / rs1 > 0.2)):
        return np.asarray(
            _reference_numpy(data, online_new, target, decoder, 0))

    SE = A[:, :, 0:9].astype(np.float64)           # [core, row, term]
    DP = A[:, :, 9:18].astype(np.float64)
    QN2 = A[:, :, 18:24].astype(np.float64)        # [core, row, qmat]
    tvals = []
    for t, (m, kv) in enumerate(TERMS):
        qn = np.sqrt(QN2[:, :, m])
        row = np.log(SE[:, :, t]) - 2.0 * DP[:, :, t] / (qn * rs1[:, kv, :])
        tvals.append(row.sum() / (N * N))
    li = (tvals[0] + tvals[1] + tvals[2]) / 3.0
    le = sum(tvals[3:9]) / 6.0
    return np.asarray(np.float32(li + le))


# revision 9
# speedup vs baseline: 1.7854x; 1.7854x over previous
"""DCMC contrastive-loss kernel for 8 Trainium2 NeuronCores (SPMD, Bass/Tile).

Row-sharded strategy (512 rows/core):
  Phase 1 — six FCNs (3 target + 3 momentum-blended online) level-major in a
            transposed [out_ch(part), row(free)] layout; full-batch BN stats via
            3 batched AllReduces; 3 decoder MLPs; L2-normalized target features
            AllGathered in d-major and row-major layouts.
  Phase 2 — per view: E = exp(20*G - 20) slab in [key(part), query(free)] layout;
            rs1 via PE-ones reduce; H = E @ fbn on PE; each contrastive term's
            exp-logit row-sum (SE) comes free from ACT accum_out during PSUM
            eviction and its DP dot is a tiny DVE op against H.
  Host    — loss_row = log(SE) - 2*dpraw/(|q|*rs1); the similarity-matrix
            "adaptive" correction is exactly zero unless an off-diagonal
            similarity exceeds ~0.3 (detected via rs1/Eii with huge margin);
            then, or when warm_up != 0, a faithful numpy fallback runs.
"""
import os
import sys

sys.path.insert(0, '/opt/trn_rl_repo')

import numpy as np
import ml_dtypes

from concourse import bacc, tile, mybir, bass_utils

BF16 = ml_dtypes.bfloat16
BF = mybir.dt.bfloat16
F32 = mybir.dt.float32
AF = mybir.ActivationFunctionType
ALU = mybir.AluOpType

N = 4096
NCORES = 8
R = N // NCORES            # 512 rows per core
DIMS = [1024, 1024, 512, 256]
NV = 3
TEMP = 0.5
BN_EPS = 1e-5
NTILES = [d // 128 for d in DIMS[1:]]   # [8, 4, 2] out tiles per level
D = DIMS[-1]               # 256 = feature dim
DK = D // 128              # 2 partition chunks of the feature dim
JT = N // 128              # 32 key tiles per view

# terms: (q_matrix_index, key_view); q-matrix 0-2 = fa_v, 3-5 = Q_v
TERMS = [(0, 0), (1, 1), (2, 2),
         (3, 1), (3, 2), (4, 0), (4, 2), (5, 0), (5, 1)]

_PROGRAM = None
LAST_EXEC_NS = None


def _build_program():
    nc = bacc.Bacc("TRN2", target_bir_lowering=False, debug=False,
                   enable_asserts=False, num_devices=NCORES)

    # ---------------- DRAM I/O ----------------
    xT = [nc.dram_tensor(f"xT{v}", [DIMS[0], R], BF, kind="ExternalInput")
          for v in range(NV)]
    W = [[nc.dram_tensor(f"w{f}_{l}", [DIMS[l], DIMS[l + 1]], BF,
                         kind="ExternalInput") for l in range(3)]
         for f in range(6)]
    gam = [nc.dram_tensor(f"gam{l}", [128, 6 * NTILES[l]], F32,
                          kind="ExternalInput") for l in range(3)]
    bet = [nc.dram_tensor(f"bet{l}", [128, 6 * NTILES[l]], F32,
                          kind="ExternalInput") for l in range(3)]
    wd1 = [nc.dram_tensor(f"wd1{v}", [D, 4 * D], BF, kind="ExternalInput")
           for v in range(NV)]
    wd2 = [nc.dram_tensor(f"wd2{v}", [4 * D, D], BF, kind="ExternalInput")
           for v in range(NV)]
    b1 = [nc.dram_tensor(f"b1{v}", [128, 8], F32, kind="ExternalInput")
          for v in range(NV)]
    b2 = [nc.dram_tensor(f"b2{v}", [128, 2], F32, kind="ExternalInput")
          for v in range(NV)]
    eye_in = nc.dram_tensor("eye_in", [128, 128], BF, kind="ExternalInput")

    outA = nc.dram_tensor("outA", [R, 24], F32, kind="ExternalOutput")
    outB = nc.dram_tensor("outB", [8, R], F32, kind="ExternalOutput")

    rg = [list(range(NCORES))]

    with tile.TileContext(nc) as tc:
        with tc.tile_pool(name="px", bufs=1) as px, \
             tc.tile_pool(name="pdram", bufs=1, space="DRAM") as pdram:
            # constants
            ones128 = px.tile([128, 1], BF, tag="ones128", name="ones128")
            nc.gpsimd.memset(ones128[:], 1.0)
            ones1f = px.tile([1, 128], F32, tag="ones1f", name="ones1f")
            nc.gpsimd.memset(ones1f[:], 1.0)
            bias_m20 = px.tile([128, 1], F32, tag="bias_m20", name="bias_m20")
            nc.gpsimd.memset(bias_m20[:], -20.0)
            bias_eps = px.tile([128, 1], F32, tag="bias_eps", name="bias_eps")
            nc.gpsimd.memset(bias_eps[:], BN_EPS)
            bias_ln2 = px.tile([128, 1], F32, tag="bias_ln2", name="bias_ln2")
            nc.gpsimd.memset(bias_ln2[:], float(np.log(2.0)))
            eye = px.tile([128, 128], BF, tag="eye", name="eye")
            nc.sync.dma_start(eye[:], eye_in[:])

            outA_t = [px.tile([128, 24], F32, tag=f"outA{i}", name=f"outA{i}") for i in range(4)]
            for i in range(4):
                nc.gpsimd.memset(outA_t[i][:], 0.0)

            # ------------- Phase 1: FCNs in two groups (target, online) ------
            X = [None] * 6          # per-FCN activations (k-major tiles)

            def fcn_group(grp):
                fcn_ids = [3 * grp, 3 * grp + 1, 3 * grp + 2]
                act_cm = tc.tile_pool(name=f"actg{grp}", bufs=1)
                act_pool = act_cm.__enter__()
                try:
                    for l in range(3):
                        ntl = NTILES[l]
                        nk = DIMS[l] // 128
                        final = (l == 2)
                        next_pool = px if final else act_pool
                        with tc.tile_pool(name=f"lvl{grp}_{l}", bufs=1) as lvl, \
                             tc.tile_pool(name=f"scr{grp}_{l}", bufs=3) as scr, \
                             tc.tile_pool(name=f"pw{grp}_{l}", bufs=DIMS[l] // 128 + 2) as pw, \
                             tc.tile_pool(name=f"pps{grp}_{l}", bufs=3,
                                          space="PSUM") as pps:
                            if l == 0:
                                x0 = {}
                                for v in range(NV):
                                    tiles = []
                                    for k in range(nk):
                                        xt = lvl.tile([128, R], BF,
                                                      tag=f"x0_{v}_{k}",
                                                      name=f"x0_{v}_{k}")
                                        nc.sync.dma_start(
                                            xt[:],
                                            xT[v][128 * k:128 * (k + 1), :])
                                        tiles.append(xt)
                                    x0[v] = tiles
                            WD = 3 * ntl
                            s12 = lvl.tile([128, 2 * WD], F32, tag="s12",
                                           name="s12")
                            yb = {}
                            for fi, f in enumerate(fcn_ids):
                                xin = x0[f % 3] if l == 0 else X[f]
                                wsl = []
                                for k in range(nk):
                                    wt = pw.tile([128, DIMS[l + 1]], BF,
                                                 tag="wsl", name="wsl")
                                    nc.sync.dma_start(
                                        wt[:], W[f][l][128 * k:128 * (k + 1), :])
                                    wsl.append(wt)
                                ybf = []
                                for m in range(ntl):
                                    ps = pps.tile([128, R], F32, tag="mm",
                                                  name="mm")
                                    for k in range(nk):
                                        nc.tensor.matmul(
                                            ps[:],
                                            wsl[k][:, 128 * m:128 * (m + 1)],
                                            xin[k][:],
                                            start=(k == 0), stop=(k == nk - 1))
                                    col = fi * ntl + m
                                    t = lvl.tile([128, R], BF,
                                                 tag=f"yb{fi}_{m}",
                                                 name=f"yb{fi}_{m}")
                                    nc.scalar.activation(
                                        t[:], ps[:], AF.Copy,
                                        accum_out=s12[:, col:col + 1])
                                    sqs = scr.tile([128, R], BF, tag="sq",
                                                   name="sq")
                                    nc.scalar.activation(
                                        sqs[:], ps[:], AF.Square,
                                        accum_out=s12[:, WD + col:WD + col + 1])
                                    ybf.append(t)
                                yb[f] = ybf
                            # batched AllReduce of this group+level's stats
                            ar_in = pdram.tile([128, 2 * WD], F32,
                                               tag=f"arin{grp}_{l}",
                                               name=f"arin{grp}_{l}")
                            ar_out = pdram.tile([128, 2 * WD], F32,
                                                tag=f"arout{grp}_{l}",
                                                name=f"arout{grp}_{l}")
                            nc.sync.dma_start(ar_in[:], s12[:])
                            nc.gpsimd.collective_compute(
                                "AllReduce", ALU.add, replica_groups=rg,
                                ins=[ar_in.opt()], outs=[ar_out.opt()])
                            s12r = lvl.tile([128, 2 * WD], F32, tag="s12r",
                                            name="s12r")
                            nc.sync.dma_start(s12r[:], ar_out[:])
                            mu = lvl.tile([128, WD], F32, tag="mu", name="mu")
                            nc.vector.tensor_scalar_mul(mu[:], s12r[:, 0:WD],
                                                        1.0 / N)
                            ex2 = lvl.tile([128, WD], F32, tag="ex2", name="ex2")
                            nc.vector.tensor_scalar_mul(ex2[:],
                                                        s12r[:, WD:2 * WD],
                                                        1.0 / N)
                            var = lvl.tile([128, WD], F32, tag="var", name="var")
                            nc.vector.tensor_tensor(out=var[:], in0=mu[:],
                                                    in1=mu[:], op=ALU.mult)
                            nc.vector.tensor_tensor(out=var[:], in0=ex2[:],
                                                    in1=var[:], op=ALU.subtract)
                            lnv = lvl.tile([128, WD], F32, tag="lnv", name="lnv")
                            nc.scalar.activation(lnv[:], var[:], AF.Ln,
                                                 bias=bias_eps[:])
                            rsd = lvl.tile([128, WD], F32, tag="rsd", name="rsd")
                            nc.scalar.activation(rsd[:], lnv[:], AF.Exp,
                                                 scale=-0.5)
                            gml = lvl.tile([128, WD], F32, tag="gml", name="gml")
                            nc.sync.dma_start(
                                gml[:], gam[l][:, WD * grp:WD * (grp + 1)])
                            btl = lvl.tile([128, WD], F32, tag="btl", name="btl")
                            nc.sync.dma_start(
                                btl[:], bet[l][:, WD * grp:WD * (grp + 1)])
                            sc = lvl.tile([128, WD], F32, tag="sc", name="sc")
                            nc.vector.tensor_tensor(out=sc[:], in0=gml[:],
                                                    in1=rsd[:], op=ALU.mult)
                            sh = lvl.tile([128, WD], F32, tag="sh", name="sh")
                            nc.vector.tensor_tensor(out=sh[:], in0=mu[:],
                                                    in1=sc[:], op=ALU.mult)
                            nc.vector.tensor_tensor(out=sh[:], in0=btl[:],
                                                    in1=sh[:], op=ALU.subtract)
                            func = AF.Relu if l < 2 else AF.Identity
                            for fi, f in enumerate(fcn_ids):
                                xo = []
                                for m in range(ntl):
                                    col = fi * ntl + m
                                    tag = (f"X{l + 1}_{f}_{m}" if not final
                                           else f"feat_{f}_{m}")
                                    t = next_pool.tile([128, R], BF, tag=tag,
                                                       name=tag)
                                    nc.scalar.activation(
                                        t[:], yb[f][m][:], func,
                                        scale=sc[:, col:col + 1],
                                        bias=sh[:, col:col + 1])
                                    xo.append(t)
                                X[f] = xo
                finally:
                    act_cm.__exit__(None, None, None)

            fcn_group(0)   # target FCNs first: their AllGathers can launch early
            fbT = {v: X[v] for v in range(NV)}     # target features (d-major)

            fbn = {}
            q_im = [[None] * 4 for _ in range(6)]
            sesc = [[None] * 4 for _ in range(6)]
            # ------- l2norm of target features, Eii, AllGathers, transposes ----
            ag1_in = pdram.tile([NV * D, R], BF, tag="ag1in", name="ag1in")
            ag1_out = pdram.tile([NCORES * NV * D, R], BF, tag="ag1out", name="ag1out")
            ag2_in = pdram.tile([R, NV * D], BF, tag="ag2in", name="ag2in")
            ag2_out = pdram.tile([N, NV * D], BF, tag="ag2out", name="ag2out")
            with tc.tile_pool(name="pns", bufs=3) as pns, \
                 tc.tile_pool(name="ppsn", bufs=1, space="PSUM") as ppsn, \
                 tc.tile_pool(name="pptr", bufs=2, space="PSUM") as pptr:
                for v in range(NV):
                    nps = ppsn.tile([1, R], F32, tag="nps", name="nps")
                    for k in range(DK):
                        sq = pns.tile([128, R], BF, tag="sqn", name="sqn")
                        nc.vector.tensor_tensor(out=sq[:], in0=fbT[v][k][:],
                                                in1=fbT[v][k][:], op=ALU.mult)
                        nc.tensor.matmul(nps[:], ones128[:], sq[:],
                                         start=(k == 0), stop=(k == DK - 1))
                    lnn = pns.tile([1, R], F32, tag="lnn", name="lnn")
                    nc.scalar.activation(lnn[:], nps[:], AF.Ln)
                    invn = pns.tile([1, R], F32, tag="invn", name="invn")
                    nc.scalar.activation(invn[:], lnn[:], AF.Exp, scale=-0.5)
                    bcp = ppsn.tile([128, R], F32, tag="bcp", name="bcp")
                    nc.tensor.matmul(bcp[:], ones1f[:], invn[:],
                                     start=True, stop=True)
                    ft = []
                    for k in range(DK):
                        t = px.tile([128, R], BF, tag=f"fbn{v}_{k}", name=f"fbn{v}_{k}")
                        nc.vector.tensor_tensor(out=t[:], in0=fbT[v][k][:],
                                                in1=bcp[:], op=ALU.mult)
                        ft.append(t)
                        nc.sync.dma_start(
                            ag1_in[D * v + 128 * k:D * v + 128 * (k + 1), :],
                            t[:])
                    fbn[v] = ft
                    # Eii = exp(20*||fbn_i||^2 - 20) for the host trigger check
                    gps = ppsn.tile([1, R], F32, tag="gps", name="gps")
                    for k in range(DK):
                        sq = pns.tile([128, R], BF, tag="sqn", name="sqn")
                        nc.vector.tensor_tensor(out=sq[:], in0=ft[k][:],
                                                in1=ft[k][:], op=ALU.mult)
                        nc.tensor.matmul(gps[:], ones128[:], sq[:],
                                         start=(k == 0), stop=(k == DK - 1))
                    eii = pns.tile([1, R], F32, tag="eii", name="eii")
                    nc.scalar.activation(eii[:], gps[:], AF.Exp, scale=20.0,
                                         bias=bias_m20[0:1, :])
                    nc.sync.dma_start(outB[3 + v:4 + v, :], eii[:])

                nc.gpsimd.collective_compute(
                    "AllGather", ALU.bypass, replica_groups=rg,
                    ins=[ag1_in.opt()], outs=[ag1_out.opt()])

                # row-major copies of fbn for the 2nd AllGather
                for v in range(NV):
                    for i in range(4):
                        fim = pns.tile([128, D], BF, tag="fim", name="fim")
                        for k in range(DK):
                            tp = pptr.tile([128, 128], BF, tag="trp", name="trp")
                            nc.tensor.transpose(
                                tp[:], fbn[v][k][:, 128 * i:128 * (i + 1)],
                                eye[:])
                            nc.scalar.activation(
                                fim[:, 128 * k:128 * (k + 1)], tp[:], AF.Copy)
                        nc.sync.dma_start(
                            ag2_in[128 * i:128 * (i + 1), D * v:D * (v + 1)],
                            fim[:])
                nc.gpsimd.collective_compute(
                    "AllGather", ALU.bypass, replica_groups=rg,
                    ins=[ag2_in.opt()], outs=[ag2_out.opt()])


            fcn_group(1)   # online FCNs
            faT = {v: X[3 + v] for v in range(NV)}  # online features (d-major)

            # ------------- MLP decoders -------------
            QT = {}
            with tc.tile_pool(name="pmlp", bufs=1) as pmlp, \
                 tc.tile_pool(name="pwd", bufs=10) as pwd, \
                 tc.tile_pool(name="ppsm", bufs=3, space="PSUM") as ppsm:
                for v in range(NV):
                    wsl = []
                    for k in range(DK):
                        wt = pwd.tile([128, 4 * D], BF, tag="wd1", name="wd1")
                        nc.sync.dma_start(wt[:],
                                          wd1[v][128 * k:128 * (k + 1), :])
                        wsl.append(wt)
                    b1t = pmlp.tile([128, 8], F32, tag=f"b1{v}", name=f"b1{v}")
                    nc.sync.dma_start(b1t[:], b1[v][:])
                    b2t = pmlp.tile([128, 2], F32, tag=f"b2{v}", name=f"b2{v}")
                    nc.sync.dma_start(b2t[:], b2[v][:])
                    h = []
                    for m in range(8):
                        ps = ppsm.tile([128, R], F32, tag="mmh", name="mmh")
                        for k in range(DK):
                            nc.tensor.matmul(
                                ps[:], wsl[k][:, 128 * m:128 * (m + 1)],
                                faT[v][k][:],
                                start=(k == 0), stop=(k == DK - 1))
                        ht = pmlp.tile([128, R], BF, tag=f"h{m}", name=f"h{m}")
                        nc.scalar.activation(ht[:], ps[:], AF.Relu,
                                             bias=b1t[:, m:m + 1])
                        h.append(ht)
                    wsl2 = []
                    for k in range(8):
                        wt = pwd.tile([128, D], BF, tag="wd2", name="wd2")
                        nc.sync.dma_start(wt[:],
                                          wd2[v][128 * k:128 * (k + 1), :])
                        wsl2.append(wt)
                    qt = []
                    for m in range(DK):
                        ps = ppsm.tile([128, R], F32, tag="mmh", name="mmh")
                        for k in range(8):
                            nc.tensor.matmul(
                                ps[:], wsl2[k][:, 128 * m:128 * (m + 1)],
                                h[k][:], start=(k == 0), stop=(k == 7))
                        t = px.tile([128, R], BF, tag=f"QT{v}_{m}", name=f"QT{v}_{m}")
                        nc.scalar.activation(t[:], ps[:], AF.Identity,
                                             bias=b2t[:, m:m + 1])
                        qt.append(t)
                    QT[v] = qt

            qmats = [faT[0], faT[1], faT[2], QT[0], QT[1], QT[2]]
            with tc.tile_pool(name="pns2", bufs=3) as pns, \
                 tc.tile_pool(name="pptr2", bufs=2, space="PSUM") as pptr:
                # i-major query copies + seScale (2/|q|)
                for m in range(6):
                    for i in range(4):
                        qim = px.tile([128, D], BF, tag=f"qim{m}_{i}", name=f"qim{m}_{i}")
                        for k in range(DK):
                            tp = pptr.tile([128, 128], BF, tag="trp", name="trp")
                            nc.tensor.transpose(
                                tp[:], qmats[m][k][:, 128 * i:128 * (i + 1)],
                                eye[:])
                            nc.scalar.activation(
                                qim[:, 128 * k:128 * (k + 1)], tp[:], AF.Copy)
                        q_im[m][i] = qim
                        sqs = pns.tile([128, D], BF, tag="sqq", name="sqq")
                        nc.scalar.activation(
                            sqs[:], qim[:], AF.Square,
                            accum_out=outA_t[i][:, 18 + m:19 + m])
                        lnq = pns.tile([128, 1], F32, tag="lnq", name="lnq")
                        nc.scalar.activation(lnq[:],
                                             outA_t[i][:, 18 + m:19 + m], AF.Ln)
                        ssc = px.tile([128, 1], F32, tag=f"sesc{m}_{i}", name=f"sesc{m}_{i}")
                        nc.scalar.activation(ssc[:], lnq[:], AF.Exp, scale=-0.5,
                                             bias=bias_ln2[:])
                        sesc[m][i] = ssc

            # ------------- Phase 2: per-view N x N work -------------
            with tc.tile_pool(name="pfk", bufs=2) as pfk, \
                 tc.tile_pool(name="pjm", bufs=4) as pjm, \
                 tc.tile_pool(name="pE", bufs=1) as pE, \
                 tc.tile_pool(name="pscr", bufs=3) as pscr, \
                 tc.tile_pool(name="ph", bufs=1) as ph, \
                 tc.tile_pool(name="ppe", bufs=2, space="PSUM") as ppe, \
                 tc.tile_pool(name="pph", bufs=1, space="PSUM") as pph, \
                 tc.tile_pool(name="ppq", bufs=3, space="PSUM") as ppq:
                for v in range(NV):
                    fkT = []
                    for k in range(DK):
                        t = pfk.tile([128, N], BF, tag=f"fkT{k}", name=f"fkT{k}")
                        for c in range(NCORES):
                            base = NV * D * c + D * v + 128 * k
                            nc.sync.dma_start(t[:, R * c:R * (c + 1)],
                                              ag1_out[base:base + 128, :])
                        fkT.append(t)
                    # E slab + rs1 + H (software-pipelined PE emission)
                    rs1_ps = pph.tile([1, R], F32, tag="rs1ps", name="rs1ps")
                    H_ps = pph.tile([128, 4 * D], F32, tag="hps", name="hps")
                    Es, jms = [None] * JT, [None] * JT

                    def red_step(jt):
                        nc.tensor.matmul(rs1_ps[:], ones128[:], Es[jt][:],
                                         start=(jt == 0), stop=(jt == JT - 1),
                                         skip_group_check=True)
                        for i in range(4):
                            nc.tensor.matmul(
                                H_ps[:, D * i:D * (i + 1)],
                                Es[jt][:, 128 * i:128 * (i + 1)], jms[jt][:],
                                start=(jt == 0), stop=(jt == JT - 1),
                                skip_group_check=True)

                    for jt in range(JT):
                        fkjm = pjm.tile([128, D], BF, tag="fkjm", name="fkjm")
                        nc.sync.dma_start(
                            fkjm[:],
                            ag2_out[128 * jt:128 * (jt + 1), D * v:D * (v + 1)])
                        jms[jt] = fkjm
                        eps_t = ppe.tile([128, R], F32, tag="eps", name="eps")
                        for k in range(DK):
                            nc.tensor.matmul(
                                eps_t[:], fkT[k][:, 128 * jt:128 * (jt + 1)],
                                fbn[v][k][:],
                                start=(k == 0), stop=(k == DK - 1))
                        E_t = pE.tile([128, R], BF, tag=f"E{jt}", name=f"E{jt}")
                        nc.scalar.activation(E_t[:], eps_t[:], AF.Exp,
                                             scale=20.0, bias=bias_m20[:])
                        Es[jt] = E_t
                        if jt > 0:
                            red_step(jt - 1)
                    red_step(JT - 1)
                    rs1_sb = pscr.tile([1, R], F32, tag="rs1sb", name="rs1sb")
                    nc.scalar.activation(rs1_sb[:], rs1_ps[:], AF.Copy)
                    nc.sync.dma_start(outB[v:v + 1, :], rs1_sb[:])
                    H_sb = []
                    for i in range(4):
                        t = ph.tile([128, D], F32, tag=f"hsb{i}", name=f"hsb{i}")
                        nc.scalar.activation(t[:], H_ps[:, D * i:D * (i + 1)],
                                             AF.Copy)
                        H_sb.append(t)
                    # q slabs
                    tlist = [(t, m) for t, (m, kv) in enumerate(TERMS)
                             if kv == v]
                    for (t_idx, m) in tlist:
                        qT = qmats[m]
                        for i in range(4):
                            seacc = pscr.tile([128, 8], F32, tag="seacc", name="seacc")
                            for jb in range(8):
                                qps = ppq.tile([128, R], F32, tag="qps", name="qps")
                                for k in range(DK):
                                    nc.tensor.matmul(
                                        qps[:],
                                        qT[k][:, 128 * i:128 * (i + 1)],
                                        fkT[k][:, R * jb:R * (jb + 1)],
                                        start=(k == 0), stop=(k == DK - 1))
                                scr2 = pscr.tile([128, R], BF, tag="scr2", name="scr2")
                                nc.scalar.activation(
                                    scr2[:], qps[:], AF.Exp,
                                    scale=sesc[m][i][:],
                                    accum_out=seacc[:, jb:jb + 1])
                            nc.vector.tensor_reduce(
                                outA_t[i][:, t_idx:t_idx + 1], seacc[:],
                                axis=mybir.AxisListType.X, op=ALU.add)
                            dtt = pscr.tile([128, D], F32, tag="dtt", name="dtt")
                            nc.vector.tensor_tensor(out=dtt[:],
                                                    in0=H_sb[i][:],
                                                    in1=q_im[m][i][:],
                                                    op=ALU.mult)
                            nc.vector.tensor_reduce(
                                outA_t[i][:, 9 + t_idx:10 + t_idx], dtt[:],
                                axis=mybir.AxisListType.X, op=ALU.add)

                zrow = pscr.tile([1, R], F32, tag="zrow", name="zrow")
                nc.gpsimd.memset(zrow[:], 0.0)
                nc.sync.dma_start(outB[6:7, :], zrow[:])
                nc.sync.dma_start(outB[7:8, :], zrow[:])
                for i in range(4):
                    nc.sync.dma_start(outA[128 * i:128 * (i + 1), :],
                                      outA_t[i][:])

    nc.compile()
    return nc


# ======================= host side =======================

def _tmap(fn, *ts):
    t0 = ts[0]
    if isinstance(t0, dict):
        return {k: _tmap(fn, *[t[k] for t in ts]) for k in t0}
    if isinstance(t0, (list, tuple)):
        return [_tmap(fn, *xs) for xs in zip(*ts)]
    return fn(*ts)


def _pack_chan(vec, ntl):
    # [128*ntl] -> [128, ntl] with channel o at [o % 128, o // 128]
    return np.ascontiguousarray(vec.reshape(ntl, 128).T.astype(np.float32))


def _prep_in_maps(data, online_new, target, decoder):
    base = {}
    for f in range(6):
        p = target[f] if f < 3 else online_new[f - 3]
        for l in range(3):
            base[f"w{f}_{l}"] = np.ascontiguousarray(p['Ws'][l].astype(BF16))
    for l in range(3):
        ntl = NTILES[l]
        g = np.empty((128, 6 * ntl), np.float32)
        b = np.empty((128, 6 * ntl), np.float32)
        for f in range(6):
            p = target[f] if f < 3 else online_new[f - 3]
            if l < 2:
                gv, bv = np.asarray(p['gammas'][l]), np.asarray(p['betas'][l])
            else:
                gv = np.ones(DIMS[3], np.float32)
                bv = np.zeros(DIMS[3], np.float32)
            g[:, f * ntl:(f + 1) * ntl] = _pack_chan(gv, ntl)
            b[:, f * ntl:(f + 1) * ntl] = _pack_chan(bv, ntl)
        base[f"gam{l}"] = g
        base[f"bet{l}"] = b
    for v in range(NV):
        dp = decoder[v]
        base[f"wd1{v}"] = np.ascontiguousarray(dp['W1'].astype(BF16))
        base[f"wd2{v}"] = np.ascontiguousarray(dp['W2'].astype(BF16))
        base[f"b1{v}"] = _pack_chan(np.asarray(dp['b1'], np.float32), 8)
        base[f"b2{v}"] = _pack_chan(np.asarray(dp['b2'], np.float32), 2)
    base["eye_in"] = np.eye(128, dtype=BF16)
    in_maps = []
    for c in range(NCORES):
        m = dict(base)
        for v in range(NV):
            sl = data[v][R * c:R * (c + 1), :]
            m[f"xT{v}"] = np.ascontiguousarray(sl.T).astype(BF16)
        in_maps.append(m)
    return in_maps


def _l2n(x):
    return x / np.maximum(np.linalg.norm(x, axis=1, keepdims=True), 1e-12)


def _reference_numpy(data, online_new, target, decoder, warm_up):
    """Faithful (f32-matrix / f64-reduction) replication of the reference."""
    def fcn(x, p):
        for i in range(2):
            y = x @ p['Ws'][i]
            mu = y.mean(0, dtype=np.float64).astype(np.float32)
            var = ((y - mu) ** 2).mean(0, dtype=np.float64).astype(np.float32)
            y = (y - mu) / np.sqrt(var + BN_EPS) * p['gammas'][i] + p['betas'][i]
            x = np.maximum(y, 0)
        y = x @ p['Ws'][2]
        mu = y.mean(0, dtype=np.float64).astype(np.float32)
        var = ((y - mu) ** 2).mean(0, dtype=np.float64).astype(np.float32)
        return (y - mu) / np.sqrt(var + BN_EPS)

    def mlp(x, p):
        return np.maximum(x @ p['W1'] + p['b1'], 0) @ p['W2'] + p['b2']

    def cal_sim(feat, temp=0.1):
        f = _l2n(feat)
        n = f.shape[0]
        euc = np.clip(2.0 - 2.0 * (f @ f.T), 0.0, None).astype(np.float32)
        sim = np.exp(-euc / np.float32(temp))
        sim = sim / sim.sum(1, keepdims=True)
        dg = np.diag(sim).copy()
        diff = np.abs(dg[:, None] - sim)
        thresh = (diff < 0.7).astype(np.float32)
        idx = np.argmin(diff + np.eye(n, dtype=np.float32), axis=1)
        possible = np.eye(n, dtype=np.float32)
        possible[np.arange(n), idx] += 1.0
        selected = thresh * possible
        sim_exp = np.exp(sim) * (1.0 - np.eye(n, dtype=np.float32))
        weight = 1.0 - sim_exp / sim_exp.sum(1, keepdims=True)
        adaptive = selected * weight + (1.0 - selected)
        return sim * adaptive

    def contrast(q, k, mask):
        logits = (_l2n(q) @ _l2n(k).T) / np.float32(TEMP)
        mx = logits.max(1, keepdims=True)
        lse = mx + np.log(np.exp(logits - mx).sum(1, keepdims=True))
        s = lse - logits
        return float((np.sum(s * mask, axis=1, dtype=np.float64)
                      / mask.sum(1, dtype=np.float64)).sum() / (N * N))

    fa = [fcn(data[v], online_new[v]) for v in range(NV)]
    Q = [mlp(fa[v], decoder[v]) for v in range(NV)]
    fb = [fcn(data[v], target[v]) for v in range(NV)]
    if warm_up:
        ne = [np.eye(N, dtype=np.float32)] * NV
    else:
        ne = [cal_sim(fb[v]) for v in range(NV)]
    li = (contrast(fa[0], fb[0], ne[0]) + contrast(fa[1], fb[1], ne[1])
          + contrast(fa[2], fb[2], ne[2])) / 3.0
    le = (contrast(Q[0], fb[1], ne[1]) + contrast(Q[0], fb[2], ne[2])
          + contrast(Q[1], fb[0], ne[0]) + contrast(Q[1], fb[2], ne[2])
          + contrast(Q[2], fb[0], ne[0]) + contrast(Q[2], fb[1], ne[1])) / 6.0
    return np.float32(li + le)


def kernel(data0, data1, data2, online_params, target_params, decoder_params,
           momentum, warm_up):
    global _PROGRAM, LAST_EXEC_NS
    data = [np.asarray(d, np.float32) for d in (data0, data1, data2)]
    f32 = lambda x: np.asarray(x, np.float32)
    online = _tmap(f32, list(online_params))
    target = _tmap(f32, list(target_params))
    decoder = _tmap(f32, list(decoder_params))
    mom = np.float32(momentum)
    online_new = _tmap(lambda a, b: mom * a + (np.float32(1) - mom) * b,
                       online, target)
    wu = int(np.asarray(warm_up))
    if wu:
        return np.asarray(
            _reference_numpy(data, online_new, target, decoder, wu))

    if _PROGRAM is None:
        _PROGRAM = _build_program()
    nc = _PROGRAM
    in_maps = _prep_in_maps(data, online_new, target, decoder)
    trace = os.environ.get("BASS_KERNEL_PROFILE", "0") == "1"
    res = bass_utils.run_bass_kernel_spmd(
        nc, in_maps, core_ids=list(range(NCORES)), trace=trace)
    LAST_EXEC_NS = res.exec_time_ns

    A = np.stack([res.results[c]["outA"] for c in range(NCORES)])  # [8,512,24]
    B = np.stack([res.results[c]["outB"] for c in range(NCORES)])  # [8,8,512]
    rs1 = B[:, 0:3, :].astype(np.float64)          # [core, view, row]
    eii = B[:, 3:6, :].astype(np.float64)
    if (not np.all(np.isfinite(A)) or not np.all(np.isfinite(B))
            or np.any(rs1 <= 0)
            or np.any((rs1 - eii) / rs1 > 0.2)):
        return np.asarray(
            _reference_numpy(data, online_new, target, decoder, 0))

    SE = A[:, :, 0:9].astype(np.float64)           # [core, row, term]
    DP = A[:, :, 9:18].astype(np.float64)
    QN2 = A[:, :, 18:24].astype(np.float64)        # [core, row, qmat]
    tvals = []
    for t, (m, kv) in enumerate(TERMS):
        qn = np.sqrt(QN2[:, :, m])
        row = np.log(SE[:, :, t]) - 2.0 * DP[:, :, t] / (qn * rs1[:, kv, :])
        tvals.append(row.sum() / (N * N))
    li = (tvals[0] + tvals[1] + tvals[2]) / 3.0
    le = sum(tvals[3:9]) / 6.0
    return np.asarray(np.float32(li + le))
